# revision 31
# baseline (speedup 1.0000x reference)
"""Trainium2 Bass kernel for nn_ContextualModel_75806172774985.

Per-sample computation (B = 4M samples, S=4 steps, Q=5 features):
    y[b, m] = sum_{s < L[b]} q0[b,s] * (A @ feats[b,s])[m],
    A = W_reg @ W_kernel  (4x4)

Memory-bound problem. Measured engine rates (fp16, per elem per partition):
DVE tensor_tensor 0.54ns (any AP shape), tensor_scalar 0.31ns,
GpSimd 1.69ns, Act 0.93ns; engines contend heavily when run concurrently,
so the design keeps DVE as the single SBUF-elementwise engine and moves
the s-summation to TensorE (PSUM traffic, not SBUF).

  - Host converts inputs to fp16, packs one dense per-partition stream:
        cin [P, 21, T]: rows 0-3  q0, rows 4-19 feats (row 4+4s+f),
                        row 20 seq_lengths. Output y [P, 4, T] fp16
        m-major; host transposes back / upcasts.
  - Tile ramp (128,384,896,...,512,256) hides DMA fill/drain; tiles
    K<=384 sum s on DVE directly (cross-engine latency > work).
  - Per tile (software-pipelined one deep):
        DVE : zm[s] = (L > s)        4x tensor_scalar (imm)
              z    = zm * q0         1x tensor_tensor
              M[4s+f] = z[s]*f[s,f]  1x tensor_tensor (bcast over f)
        PE  : c = sum_s M[4s:4s+4]   4 accumulating fp16 identity matmuls
              per 128-sample group -> PSUM (f-major [4,128] per group)
        Act : c PSUM -> SBUF fp16 copy per group
        DVE : t[4m+f] = A[m,f]*c[f]  8x tensor_scalar (f=0,1)
        Act : t rows for f=2,3       8x activation (scale = AP)
        DVE : y[m] = sum_f t[4m+f]   2x slab tensor_tensor adds
    A is computed on device once (tiny ones-matmul broadcast into a_sb).
"""
import numpy as np
from concurrent.futures import ThreadPoolExecutor

import concourse.bass as bass
import concourse.tile as tile
from concourse import bacc, mybir
from concourse.bass_utils import run_bass_kernel_spmd

N_CORES = 8
P = 128
B_TOTAL = 4_000_000
BS = B_TOTAL // N_CORES          # 500_000 samples per core
T = 3968                         # samples per partition (128*3968 = 507904)
BSP = P * T
GRP = 128                        # samples per PSUM group (512 psum cols)

f32 = mybir.dt.float32
f16 = mybir.dt.float16

K_TILES = (128, 384, 896, 896, 896, 512, 256)
DVE_C_MAX = 384                  # tiles this small sum s on DVE, no PE/Act
ACT_F = (2, 3)                   # A-mul rows t[4m+f] for these f run on Act
PE_TAIL = 512                    # per big tile, trailing cols whose f-sum
                                 # runs on PE -> PSUM -> f32 DMA (host merge)


def _pe_ranges():
    out = []
    base = 0
    for K in K_TILES:
        kp = PE_TAIL if K > DVE_C_MAX else 0
        out.append((base, K - kp, K))
        base += K
    return out


def build_nc(num_devices=N_CORES):
    assert sum(K_TILES) == T
    for k in K_TILES:
        assert k % GRP == 0
    nc = bacc.Bacc("TRN2", target_bir_lowering=False, debug=False,
                   enable_asserts=False, num_devices=num_devices)

    c_d = nc.dram_tensor("cin", [P, 21 * T], f16, kind="ExternalInput")
    wk_d = nc.dram_tensor("w_kernel", [4, 4], f32, kind="ExternalInput")
    wr_d = nc.dram_tensor("w_reg", [4, 4], f32, kind="ExternalInput")
    y_d = nc.dram_tensor("y", [P, 4 * T], f16, kind="ExternalOutput")

    ones_np = np.ones((4, 128), dtype=np.float32)
    ones_d = nc.inline_tensor(ones_np, name="ones4x128")
    ident_np = np.eye(128, dtype=np.float16)
    ident_d = nc.inline_tensor(ident_np, name="ident128f16")

    c_h = c_d.ap().rearrange("p (r t) -> p r t", r=21)       # [128, 21, T]
    y_h = y_d.ap().rearrange("p (m t) -> p m t", m=4)        # [128, 4, T]

    mult = mybir.AluOpType.mult
    add = mybir.AluOpType.add
    is_gt = mybir.AluOpType.is_gt

    with tile.TileContext(nc) as tc:
        with (
            tc.tile_pool(name="cin", bufs=2) as in_pool,
            tc.tile_pool(name="mp", bufs=2) as m_pool,
            tc.tile_pool(name="cp", bufs=3) as c_pool,
            tc.tile_pool(name="work", bufs=1) as work,
            tc.tile_pool(name="yout", bufs=2) as y_pool,
            tc.tile_pool(name="singles", bufs=1) as singles,
            tc.tile_pool(name="ps", bufs=1, space="PSUM") as ps_pool,
            tc.tile_pool(name="psg", bufs=4, space="PSUM") as psg_pool,
            tc.tile_pool(name="psy", bufs=2, space="PSUM") as psy_pool,
        ):
            # ---- one-time: a_sb[p, 4f+m] = A[m,f] = sum_c Wreg[m,c]Wkern[c,f]
            wrT = singles.tile([4, 4], f32)
            nc.scalar.dma_start(out=wrT[:], in_=wr_d.ap().transpose([1, 0]))
            wk_s = singles.tile([4, 4], f32)
            nc.scalar.dma_start(out=wk_s[:], in_=wk_d.ap())
            ones_sb = singles.tile([4, 128], f32)
            nc.scalar.dma_start(out=ones_sb[:], in_=ones_d.ap())
            ident = singles.tile([128, 128], f16)
            nc.scalar.dma_start(out=ident[:], in_=ident_d.ap())

            wrT_rep = bass.AP(tensor=wrT.tensor, offset=wrT.offset,
                              ap=[list(wrT.ap[0]), [0, 4], [1, 4]])
            wk_rep = bass.AP(tensor=wk_s.tensor, offset=wk_s.offset,
                             ap=[list(wk_s.ap[0]), [1, 4], [0, 4]])
            r_sb = singles.tile([4, 4, 4], f32)
            nc.vector.tensor_tensor(out=r_sb[:], in0=wrT_rep, in1=wk_rep,
                                    op=mult)
            a_ps = ps_pool.tile([128, 16], f32)
            nc.tensor.matmul(a_ps[:], ones_sb[:],
                             r_sb.rearrange("c f m -> c (f m)"))
            a_sb = singles.tile([128, 16], f32)
            nc.scalar.copy(a_sb[:], a_ps[:])

            # ---- work tiles ----
            kmax = max(K_TILES)
            z_w = work.tile([P, 4, kmax], f16)
            t_w = work.tile([P, 16, kmax], f16)

            # ---- main loop, software-pipelined one tile deep ----
            pend = None              # (c_sb, yt, K, base) awaiting A-stage

            def a_stage(c_sb, yt, K, tbase):
                kp = min(PE_TAIL, K - GRP) if K > DVE_C_MAX else 0
                kd = K - kp
                tk = t_w.rearrange("p (m f) k -> p m f k", m=4)[:, :, :, :K]
                for m in range(4):
                    for f in range(4):
                        if f in ACT_F:
                            nc.scalar.activation(
                                out=tk[:, m, f, :], in_=c_sb[:, f, :K],
                                func=mybir.ActivationFunctionType.Copy,
                                scale=a_sb[:, 4 * f + m:4 * f + m + 1])
                        else:
                            nc.vector.tensor_scalar(
                                out=tk[:, m, f, :], in0=c_sb[:, f, :K],
                                scalar1=a_sb[:, 4 * f + m:4 * f + m + 1],
                                scalar2=None, op0=mult)
                nc.vector.tensor_tensor(out=tk[:, :, 0:2, :kd],
                                        in0=tk[:, :, 0:2, :kd],
                                        in1=tk[:, :, 2:4, :kd], op=add)
                nc.vector.tensor_tensor(out=yt[:, :, :kd],
                                        in0=tk[:, :, 0, :kd],
                                        in1=tk[:, :, 1, :kd], op=add)
                # PE f-sum for the trailing kp cols; Act stages to fp16
                for g in range(kp // GRP):
                    gs = slice(kd + g * GRP, kd + (g + 1) * GRP)
                    y_ps = psy_pool.tile([128, 4, GRP], f32, tag="yps")
                    for f in range(4):
                        nc.tensor.matmul(y_ps[:], ident[:],
                                         tk[:, :, f, gs],
                                         start=(f == 0), stop=(f == 3))
                    nc.scalar.copy(yt[:, :, gs], y_ps[:])
                nc.gpsimd.dma_start(out=y_h[:, :, tbase:tbase + K],
                                    in_=yt[:])

            base = 0
            for K in K_TILES:
                ct = in_pool.tile([P, 21, K], f16)
                nc.sync.dma_start(out=ct[:], in_=c_h[:, :, base:base + K])
                yt = y_pool.tile([P, 4, K], f16)
                mt = m_pool.tile([P, 16, K], f16, tag="mt")
                c_sb = c_pool.tile([P, 4, K], f16, tag="csb")

                # DVE: z = (L > s) * q0
                z = z_w[:, :, :K]
                for s in range(4):
                    nc.vector.tensor_scalar(
                        out=z[:, s, :], in0=ct[:, 20, :], scalar1=float(s),
                        scalar2=None, op0=is_gt)
                nc.vector.tensor_tensor(out=z, in0=z, in1=ct[:, 0:4, :],
                                        op=mult)
                # DVE: M[4s+f] = z[s] * feats[s,f]
                fts = ct[:, 4:20, :].rearrange("p (s f) k -> p s f k", s=4)
                msf = mt.rearrange("p (s f) k -> p s f k", s=4)
                h = K // 2
                for lo, hi in ((0, h), (h, K)):
                    zbc = z[:, :, lo:hi].unsqueeze(2).broadcast_to(
                        [P, 4, 4, hi - lo])
                    nc.vector.tensor_tensor(out=msf[:, :, :, lo:hi],
                                            in0=zbc,
                                            in1=fts[:, :, :, lo:hi],
                                            op=mult)

                # A-stage of the previous tile first: its Act rows must
                # not queue behind this tile's PSUM copies on Act
                if pend is not None:
                    a_stage(*pend)

                if K <= DVE_C_MAX:
                    # small tile: sum s on DVE (cross-engine latency > work)
                    nc.vector.tensor_tensor(out=msf[:, 0:2], in0=msf[:, 0:2],
                                            in1=msf[:, 2:4], op=add)
                    nc.vector.tensor_tensor(out=c_sb[:], in0=msf[:, 0],
                                            in1=msf[:, 1], op=add)
                else:
                    # PE/Act: c[f] = sum_s M[s,f], per 128-sample group
                    for g in range(K // GRP):
                        gs = slice(g * GRP, (g + 1) * GRP)
                        c_ps = psg_pool.tile([128, 4, GRP], f32, tag="cps")
                        for s in range(4):
                            nc.tensor.matmul(c_ps[:], ident[:],
                                             msf[:, s, :, gs],
                                             start=(s == 0), stop=(s == 3))
                        nc.scalar.copy(c_sb[:, :, gs], c_ps[:])

                pend = (c_sb, yt, K, base)
                base += K

            a_stage(*pend)
    nc.compile()
    return nc


_NC_CACHE = None


def _get_nc():
    global _NC_CACHE
    if _NC_CACHE is None:
        _NC_CACHE = build_nc()
    return _NC_CACHE


def _prep_core(args):
    xss, seq, c = args
    x = np.zeros((BSP, 4, 5), np.float16)
    x[:BS] = xss[c * BS:(c + 1) * BS]
    lp = np.zeros((BSP,), np.float16)
    lp[:BS] = seq[c * BS:(c + 1) * BS]
    arr = x.reshape(P, T, 4, 5)
    cin = np.empty((P, 21, T), np.float16)
    cin[:, 0:4] = arr[:, :, :, 0].transpose(0, 2, 1)
    cin[:, 4:20] = arr[:, :, :, 1:].transpose(0, 2, 3, 1).reshape(P, 16, T)
    cin[:, 20] = lp.reshape(P, T)
    return {"cin": cin.reshape(P, 21 * T)}


def _shard_inputs(xss, seq_lengths, W_kernel, W_reg):
    xss = np.asarray(xss, dtype=np.float32).reshape(B_TOTAL, 4, 5)
    seq = np.asarray(seq_lengths)
    wk = np.ascontiguousarray(W_kernel, dtype=np.float32)
    wr = np.ascontiguousarray(W_reg, dtype=np.float32)
    with ThreadPoolExecutor(N_CORES) as ex:
        maps = list(ex.map(_prep_core,
                           [(xss, seq, c) for c in range(N_CORES)]))
    for m in maps:
        m["w_kernel"] = wk
        m["w_reg"] = wr
    return maps


def run(xss, seq_lengths, W_kernel, W_reg, trace=False, **spmd_kwargs):
    nc = _get_nc()
    in_maps = _shard_inputs(xss, seq_lengths, W_kernel, W_reg)
    last_err = None
    for _attempt in range(3):
        try:
            res = run_bass_kernel_spmd(nc, in_maps,
                                       core_ids=list(range(N_CORES)),
                                       trace=trace, **spmd_kwargs)
            break
        except Exception as e:  # transient NRT_EXEC_UNIT_UNRECOVERABLE
            last_err = e
    else:
        raise last_err

    def _post(r):
        y = r["y"].reshape(P, 4, T).transpose(0, 2, 1).reshape(BSP, 4)
        return y[:BS].astype(np.float32)

    with ThreadPoolExecutor(N_CORES) as ex:
        parts = list(ex.map(_post, res.results))
    out = np.concatenate(parts, axis=0)
    return out, res


def kernel(xss, seq_lengths, W_kernel, W_reg):
    out, _ = run(xss, seq_lengths, W_kernel, W_reg)
    return out
